# revision 47
# baseline (speedup 1.0000x reference)
"""Trainium2 Bass kernel for nn_CustomLlamaAttention (B=2, S=2048, D=2048, H=16).

Sharding: batch*heads across 8 cores -> each core owns 2 heads x 2 batches.
Wq/Wk/Wv split column-wise (by head) per core; Wo split row-wise; each core
computes a partial [B,S,D] output (bf16) which the host sums in float64.

Per-core dataflow, all-bf16 operands (PSUM accumulation is fp32):
  QT/KT  [hd=128, S] = (Wq shard)^T-tiles (stationary) x X^T (moving)
  V      [S, hd] natural = X^T-tiles (stationary) x Wv^T (moving)
  RoPE: rotate_half is a signed 128x128 permutation matmul on the PE
        (prot = R^T q), then 3 DVE ops: q' = q*cos + prot*sin.
  scoresT[sk, sq] pairs: two 128-row score matmuls write one PSUM tile
        [128, 2, 512]; ONE wide ScalarE exp per pair -> p bf16 (amortizes
        the ACT engine's fixed ~352-cycle overhead so exp stays off the
        PE critical path). Attention runs as one flat software pipeline
        over (qi, h, pair) so the exp latency is only exposed once/batch.
  uoutT [hd, sq] += V-tile (stationary) x expT (moving)
  rowsums via ones-matmul; aT = uoutT * (1/rowsums) -> bf16
  output proj: half-D rows accumulate in PSUM, ONE wide copy (mostly
        ScalarE; DVE takes every 3rd row-block once its attention tail
        has drained) + ONE DMA per half on the sync queue; the wo phase
        of batch b is emitted inside batch b+1's projection pool scope
        so no PSUM pool boundary stalls the PE between them.

Weights/xt are loaded as ETQ-row groups (tile per group) so the first
matmuls only wait on the first group's DMA. PSUM pools are phase-scoped
per batch (proj+wo 16KB / attn 16KB).
"""

import sys

for _p in ("/opt/trn_rl_repo", "/opt/trn_rl_repo/concourse"):
    if _p not in sys.path:
        sys.path.insert(0, _p)

import math

import ml_dtypes
import numpy as np

# ---------------------------------------------------------------- config
N_CORES = 8
NUM_HEADS = 16
ROPE_BASE = 10000.0
HD = 128  # head dim

MM_DT = "bfloat16"  # kept for test.py compat; kernel is bf16-only

_CACHE = {}

BF16 = ml_dtypes.bfloat16


def _full_cfg():
    return dict(B=2, S=2048, D=2048, NH=NUM_HEADS // N_CORES)


# ---------------------------------------------------------------- device program
def build_core_program(B, S, D, NH, mm_dt_name=None):
    """Build the single-core Bass program (identical on all 8 cores)."""
    import concourse.mybir as mybir
    from concourse import bacc
    from concourse.tile import TileContext

    f32 = mybir.dt.float32
    bf = mybir.dt.bfloat16

    hd = HD
    DQ = NH * hd           # per-core projection width (256)
    ET = D // 128          # contraction tiles over model dim
    SC = min(512, S)       # s-chunk width in projection phase (moving max)
    NSC = S // SC
    SBK = SC // 128        # s-blocks per chunk (for V)
    RU = min(512, S)       # RoPE unit width
    CPU = RU // SC         # chunks per rope unit
    SQT = min(512, S)      # attention sq tile width
    NSQ = S // SQT
    SKB = S // 128         # sk blocks
    NPAIR = SKB // 2
    SB = S // 128
    EOW = min(512, D)      # output-proj matmul width
    HW_ = D // 2           # output row half-width
    EOW2 = min(EOW, HW_)
    NEO2 = HW_ // EOW2
    inv_sqrt_hd = 1.0 / math.sqrt(hd)

    nc = bacc.Bacc(trn_type="TRN2", target_bir_lowering=False)

    xt = nc.dram_tensor("xt", [B, ET, 128, S], bf, kind="ExternalInput")
    wq = nc.dram_tensor("wq", [ET, 128, DQ], bf, kind="ExternalInput")
    wk = nc.dram_tensor("wk", [ET, 128, DQ], bf, kind="ExternalInput")
    wv = nc.dram_tensor("wv", [ET, 128, DQ], bf, kind="ExternalInput")
    wo = nc.dram_tensor("wo", [NH, 128, D], bf, kind="ExternalInput")
    cos = nc.dram_tensor("cos", [128, S], bf, kind="ExternalInput")
    sin = nc.dram_tensor("sin", [128, S], bf, kind="ExternalInput")
    rotm = nc.dram_tensor("rotm", [128, 128], bf, kind="ExternalInput")
    out = nc.dram_tensor("out", [B, SB, 128, D], bf, kind="ExternalOutput")

    Exp = mybir.ActivationFunctionType.Exp

    with TileContext(nc) as tc:
        with tc.tile_pool(name="sb", bufs=1) as sbp:
            # ---------- resident constants; weights/xt split into ETQ-row
            # groups (tile per group) to shorten first-matmul DMA waits
            ETQ = max(1, ET // 4)
            NG = ET // ETQ

            def wgroup_tiles(nm):
                return [
                    sbp.tile([128, ETQ, DQ], bf, name=f"{nm}_sb{g}")
                    for g in range(NG)
                ]

            wq_sb = wgroup_tiles("wq")
            wk_sb = wgroup_tiles("wk")
            wv_sb = wgroup_tiles("wv")
            wo_sb = sbp.tile([128, NH, D], bf, name="wo_sb")
            cos_sb = sbp.tile([128, S], bf, name="cos_sb")
            sin_sb = sbp.tile([128, S], bf, name="sin_sb")
            rot_sb = sbp.tile([128, 128], bf, name="rot_sb")
            ones_sb = sbp.tile([128, 128], bf, name="ones_sb")

            def load_xt_chunk(b, c):
                csl = slice(c * SC, (c + 1) * SC)
                tiles = []
                for g in range(NG):
                    t_sb = sbp.tile(
                        [128, ETQ, SC], bf, tag=f"xt{g}",
                        name=f"xt_{b}_{c}_{g}", bufs=2,
                    )
                    nc.sync.dma_start(
                        t_sb[:],
                        xt[b, g * ETQ : (g + 1) * ETQ, :, csl].rearrange(
                            "t p s -> p t s"
                        ),
                    )
                    tiles.append(t_sb)
                return tiles

            # All constant loads go on the sync queue: the Scalar queue must
            # stay free for the first projection copies (each DMA issue
            # costs ~0.6us of queue time)
            xt_next = load_xt_chunk(0, 0)
            # Q/K weights first (consumed from the first matmul); V weights
            # after (first needed at the end of chunk 0)
            for g in range(NG):
                for w_dram, w_tiles in ((wq, wq_sb), (wk, wk_sb)):
                    nc.scalar.dma_start(
                        w_tiles[g][:],
                        w_dram[g * ETQ : (g + 1) * ETQ].rearrange(
                            "t p d -> p t d"
                        ),
                    )
            for g in range(NG):
                nc.sync.dma_start(
                    wv_sb[g][:],
                    wv[g * ETQ : (g + 1) * ETQ].rearrange("t p d -> p t d"),
                )
            nc.sync.dma_start(cos_sb[:], cos[:])
            nc.sync.dma_start(sin_sb[:], sin[:])
            nc.sync.dma_start(rot_sb[:], rotm[:])
            ones_f32 = sbp.tile([128, 128], f32, name="ones_f32")
            nc.vector.memset(ones_f32[:], 1.0)
            nc.vector.tensor_copy(ones_sb[:], ones_f32[:])
            nc.sync.dma_start(wo_sb[:], wo[:].rearrange("h p e -> p h e"))

            def emit_wo(bb, at_list, pwp, pw_tag="pw", pw_bufs=2):
                # output projection for batch bb: half-D rows ([128, D/2]
                # PSUM accum -> ONE wide copy -> ONE DMA per half. Runs in
                # the attention scope, reusing the freed score-PSUM slots
                # (same 4KB slot size).
                for sb_i in range(SB):
                    for half in range(2):
                        pw = pwp.tile(
                            [128, HW_], f32, tag=pw_tag, bufs=pw_bufs
                        )
                        for eo in range(NEO2):
                            eosl = slice(
                                half * HW_ + eo * EOW2,
                                half * HW_ + (eo + 1) * EOW2,
                            )
                            qi_i = (sb_i * 128) // SQT
                            qoff = sb_i * 128 - qi_i * SQT
                            for a_t in range(NH):
                                nc.tensor.matmul(
                                    pw[:, eo * EOW2 : (eo + 1) * EOW2],
                                    at_list[a_t][qi_i][
                                        :, qoff : qoff + 128
                                    ],
                                    wo_sb[:, a_t, eosl],
                                    start=(a_t == 0),
                                    stop=(a_t == NH - 1),
                                )
                        osb = sbp.tile(
                            [128, HW_], bf, tag="osb", name="osb", bufs=6
                        )
                        # copies lean on Scalar (2 of 3): the DVE still
                        # drains the attention tail (recips/at-muls) when
                        # the wo phase starts
                        if half == 0 or sb_i < 5 or sb_i % 3 != 2:
                            nc.scalar.copy(osb[:], pw[:])
                        else:
                            nc.vector.tensor_copy(osb[:], pw[:])
                        nc.sync.dma_start(
                            out[bb, sb_i, :, half * HW_ : (half + 1) * HW_],
                            osb[:],
                        )

            at_prev = None
            for b in range(B):
                # ---------- projections + RoPE for batch b
                qt = [
                    sbp.tile([128, S], bf, tag=f"q{h}", name=f"qt{h}_{b}")
                    for h in range(NH)
                ]
                kt = [
                    sbp.tile([128, S], bf, tag=f"k{h}", name=f"kt{h}_{b}")
                    for h in range(NH)
                ]
                # V split into quarters to shorten tile-level dep chains
                SBV = max(1, SB // 4)
                v_sb = [
                    sbp.tile(
                        [128, SBV, DQ], bf, tag=f"v{g}", name=f"v{g}_{b}"
                    )
                    for g in range(SB // SBV)
                ]

                with tc.tile_pool(name=f"pj{b}", bufs=1, space="PSUM") as pjp:
                    # previous batch's output projection shares this pool
                    # scope so no PSUM pool boundary stalls the PE
                    if at_prev is not None:
                        emit_wo(b - 1, at_prev, pjp)
                    raw = {}  # (h, 0=q/1=k) -> staging tile for current unit
                    for c in range(NSC):
                        csl_u = slice((c % CPU) * SC, (c % CPU + 1) * SC)
                        xt_sb = xt_next
                        nxt = (b, c + 1) if c + 1 < NSC else (b + 1, 0)
                        if nxt[0] < B:
                            xt_next = load_xt_chunk(*nxt)
                        if c % CPU == 0:
                            for h in range(NH):
                                raw[(h, 0)] = sbp.tile(
                                    [128, RU], bf, tag=f"qr{h}",
                                    name=f"qr{h}", bufs=2,
                                )
                                raw[(h, 1)] = sbp.tile(
                                    [128, RU], bf, tag=f"kr{h}",
                                    name=f"kr{h}", bufs=2,
                                )
                        for h in range(NH):
                            for i, w_sb in enumerate((wq_sb, wk_sb)):
                                ps = pjp.tile(
                                    [128, SC], f32, tag="pj", name="ps",
                                    bufs=2,
                                )
                                for t in range(ET):
                                    nc.tensor.matmul(
                                        ps[:],
                                        w_sb[t // ETQ][
                                            :, t % ETQ, h * hd : (h + 1) * hd
                                        ],
                                        xt_sb[t // ETQ][:, t % ETQ, :],
                                        start=(t == 0),
                                        stop=(t == ET - 1),
                                    )
                                nc.scalar.copy(raw[(h, i)][:, csl_u], ps[:])
                        for s2 in range(SBK):
                            psv = pjp.tile(
                                [128, DQ], f32, tag="pj", name="ps", bufs=2
                            )
                            for t in range(ET):
                                nc.tensor.matmul(
                                    psv[:],
                                    xt_sb[t // ETQ][
                                        :, t % ETQ, s2 * 128 : (s2 + 1) * 128
                                    ],
                                    wv_sb[t // ETQ][:, t % ETQ, :],
                                    start=(t == 0),
                                    stop=(t == ET - 1),
                                )
                            cb = c * SBK + s2
                            nc.scalar.copy(
                                v_sb[cb // SBV][:, cb % SBV, :], psv[:]
                            )
                        if (c + 1) % CPU == 0:
                            u = (c + 1) // CPU - 1
                            usl = slice(u * RU, (u + 1) * RU)
                            for h in range(NH):
                                for i, dst in ((0, qt[h]), (1, kt[h])):
                                    src = raw[(h, i)]
                                    prot = pjp.tile(
                                        [128, RU], f32, tag="rot",
                                        name="prot", bufs=2,
                                    )
                                    nc.tensor.matmul(
                                        prot[:], rot_sb[:], src[:],
                                        start=True, stop=True,
                                    )
                                    tsin = sbp.tile(
                                        [128, RU], bf, tag="tsin",
                                        name="tsin", bufs=2,
                                    )
                                    nc.vector.tensor_mul(
                                        tsin[:], prot[:], sin_sb[:, usl]
                                    )
                                    tcos = sbp.tile(
                                        [128, RU], bf, tag="tcos",
                                        name="tcos", bufs=2,
                                    )
                                    nc.vector.tensor_mul(
                                        tcos[:], src[:], cos_sb[:, usl]
                                    )
                                    nc.vector.tensor_add(
                                        dst[:, usl], tcos[:], tsin[:]
                                    )

                # ---------- attention: flat pipeline over (qi, h, pair)
                # at split per (h, qi) so early wo row-blocks only wait
                # on their own qi's normalize, not the whole attention drain
                at = [
                    [
                        sbp.tile(
                            [128, SQT], bf, tag=f"a{h}_{qi}",
                            name=f"at{h}_{qi}_{b}",
                        )
                        for qi in range(NSQ)
                    ]
                    for h in range(NH)
                ]
                with tc.tile_pool(name=f"at{b}", bufs=1, space="PSUM") as app:
                    tasks = [(qi, h) for qi in range(NSQ) for h in range(NH)]
                    allpairs = [
                        (ti, j)
                        for ti in range(len(tasks))
                        for j in range(NPAIR)
                    ]

                    def score_pair(ti, j):
                        qi, h = tasks[ti]
                        sq = slice(qi * SQT, (qi + 1) * SQT)
                        sc_t = app.tile(
                            [128, 2, SQT], f32, tag="sc",
                            name=f"sc{ti}_{j}", bufs=2,
                        )
                        for i in range(2):
                            ki = 2 * j + i
                            nc.tensor.matmul(
                                sc_t[:, i, :],
                                kt[h][:, ki * 128 : (ki + 1) * 128],
                                qt[h][:, sq],
                                start=True,
                                stop=True,
                            )
                        p_sb = sbp.tile(
                            [128, 2, SQT], bf, tag="p",
                            name=f"p{ti}_{j}", bufs=4,
                        )
                        nc.scalar.activation(
                            p_sb[:], sc_t[:], Exp, scale=inv_sqrt_hd
                        )
                        return p_sb

                    # prefetch depth 2: the first two pairs' scores+exp
                    # are issued before any AV consumes, so the ACT queue
                    # is primed past the projection-tail copies
                    po = pr = None
                    DEPTH = 3
                    pend = [
                        score_pair(*allpairs[k])
                        for k in range(min(DEPTH, len(allpairs)))
                    ]
                    for idx, (ti, j) in enumerate(allpairs):
                        qi, h = tasks[ti]
                        sq = slice(qi * SQT, (qi + 1) * SQT)
                        p_sb = pend.pop(0)
                        if idx + DEPTH < len(allpairs):
                            pend.append(score_pair(*allpairs[idx + DEPTH]))
                        if j == 0:
                            po = app.tile(
                                [128, SQT], f32, tag="oc", name="po", bufs=2
                            )
                            pr = app.tile(
                                [128, SQT], f32, tag="rc", name="pr", bufs=2
                            )
                        for i in range(2):
                            ki = 2 * j + i
                            nc.tensor.matmul(
                                po[:],
                                v_sb[ki // SBV][
                                    :, ki % SBV, h * hd : (h + 1) * hd
                                ],
                                p_sb[:, i, :],
                                start=(ki == 0),
                                stop=(ki == SKB - 1),
                            )
                            nc.tensor.matmul(
                                pr[:],
                                ones_sb[:],
                                p_sb[:, i, :],
                                start=(ki == 0),
                                stop=(ki == SKB - 1),
                            )
                        if j == NPAIR - 1:
                            r_sb = sbp.tile(
                                [128, SQT], f32, tag="r", name="r_sb", bufs=2
                            )
                            nc.vector.reciprocal_approx_fast(
                                out=r_sb[:], in_=pr[:]
                            )
                            nc.vector.tensor_mul(
                                at[h][qi][:], po[:], r_sb[:]
                            )

                at_prev = at

            # final batch's output projection (whole PSUM is free here)
            with tc.tile_pool(name="wolast", bufs=1, space="PSUM") as pwp:
                emit_wo(B - 1, at_prev, pwp, pw_bufs=4)

    nc.compile()
    return nc


# ---------------------------------------------------------------- host helpers
def _rope_tables(S, dtype=BF16):
    """cos/sin tables [128, S] in [d, s] layout (plain sin; sign lives in
    the rotation matrix)."""
    inv_freq = 1.0 / (ROPE_BASE ** (np.arange(0, HD, 2, dtype=np.float32) / HD))
    t = np.arange(S, dtype=np.float32)
    freqs = np.outer(t, inv_freq)  # [S, half]
    cos = np.cos(freqs).T  # [half, S]
    sin = np.sin(freqs).T
    cosT = np.concatenate([cos, cos], axis=0).astype(dtype)  # [128, S]
    sinT = np.concatenate([sin, sin], axis=0).astype(dtype)
    return np.ascontiguousarray(cosT), np.ascontiguousarray(sinT)


def _rot_matrix(dtype=BF16):
    """Signed permutation R [128,128] (stationary layout) s.t.
    (R^T q)[i] = rotate_half(q)[i] for q in [d, s] layout."""
    half = HD // 2
    m = np.zeros((HD, HD), dtype=np.float32)
    for i in range(HD):
        m[(i + half) % HD, i] = -1.0 if i < half else 1.0
    return np.ascontiguousarray(m.astype(dtype))


def _prep_inputs(hidden_states, Wq, Wk, Wv, Wo, cfg, n_cores=N_CORES):
    """Build the per-core input dicts (all bf16)."""
    B, S, D, NH = cfg["B"], cfg["S"], cfg["D"], cfg["NH"]
    ET = D // 128
    DQ = NH * HD

    x = np.asarray(hidden_states, dtype=np.float32)
    xt = (
        np.ascontiguousarray(x.transpose(0, 2, 1))
        .astype(BF16)
        .reshape(B, ET, 128, S)
    )
    cosT, sinT = _rope_tables(S)
    rotmat = _rot_matrix()

    in_maps = []
    for c in range(n_cores):
        lo, hi = c * DQ, (c + 1) * DQ
        wq_c = np.ascontiguousarray(np.asarray(Wq)[lo:hi, :].T).astype(BF16)
        wk_c = np.ascontiguousarray(np.asarray(Wk)[lo:hi, :].T).astype(BF16)
        wv_c = np.ascontiguousarray(np.asarray(Wv)[lo:hi, :].T).astype(BF16)
        wo_c = np.ascontiguousarray(np.asarray(Wo)[:, lo:hi].T).astype(BF16)
        in_maps.append(
            {
                "xt": xt,
                "wq": wq_c.reshape(ET, 128, DQ),
                "wk": wk_c.reshape(ET, 128, DQ),
                "wv": wv_c.reshape(ET, 128, DQ),
                "wo": wo_c.reshape(NH, 128, D),
                "cos": cosT,
                "sin": sinT,
                "rotm": rotmat,
            }
        )
    return in_maps


def _gather(results, cfg):
    B, S, D = cfg["B"], cfg["S"], cfg["D"]
    acc = np.zeros((B, S, D), dtype=np.float64)
    for r in results:
        acc += np.asarray(r["out"]).astype(np.float64).reshape(B, S, D)
    return acc.astype(np.float32)


# ---------------------------------------------------------------- entry point
def kernel(hidden_states, Wq, Wk, Wv, Wo):
    from concourse.bass_utils import run_bass_kernel_spmd

    cfg = _full_cfg()
    key = ("nc", cfg["B"], cfg["S"], cfg["D"], cfg["NH"])
    if key not in _CACHE:
        _CACHE[key] = build_core_program(cfg["B"], cfg["S"], cfg["D"], cfg["NH"])
    nc = _CACHE[key]

    in_maps = _prep_inputs(hidden_states, Wq, Wk, Wv, Wo, cfg)
    res = run_bass_kernel_spmd(nc, in_maps, core_ids=list(range(N_CORES)))
    return _gather(res.results, cfg)


# revision 48
# speedup vs baseline: 1.0184x; 1.0184x over previous
"""Trainium2 Bass kernel for nn_CustomLlamaAttention (B=2, S=2048, D=2048, H=16).

Sharding: batch*heads across 8 cores -> each core owns 2 heads x 2 batches.
Wq/Wk/Wv split column-wise (by head) per core; Wo split row-wise; each core
computes a partial [B,S,D] output (bf16) which the host sums in float64.

Per-core dataflow, all-bf16 operands (PSUM accumulation is fp32):
  QT/KT  [hd=128, S] = (Wq shard)^T-tiles (stationary) x X^T (moving)
  V      [S, hd] natural = X^T-tiles (stationary) x Wv^T (moving)
  RoPE: rotate_half is a signed 128x128 permutation matmul on the PE
        (prot = R^T q), then 3 DVE ops: q' = q*cos + prot*sin.
  scoresT[sk, sq] pairs: two 128-row score matmuls write one PSUM tile
        [128, 2, 512]; ONE wide ScalarE exp per pair -> p bf16 (amortizes
        the ACT engine's fixed ~352-cycle overhead so exp stays off the
        PE critical path). Attention runs as one flat software pipeline
        over (qi, h, pair) so the exp latency is only exposed once/batch.
  uoutT [hd, sq] += V-tile (stationary) x expT (moving)
  rowsums via ones-matmul; aT = uoutT * (1/rowsums) -> bf16
  output proj: half-D rows accumulate in PSUM, ONE wide copy (mostly
        ScalarE; DVE takes every 3rd row-block once its attention tail
        has drained) + ONE DMA per half on the sync queue; the wo phase
        of batch b is emitted inside batch b+1's projection pool scope
        so no PSUM pool boundary stalls the PE between them.

Weights/xt are loaded as ETQ-row groups (tile per group) so the first
matmuls only wait on the first group's DMA. PSUM pools are phase-scoped
per batch (proj+wo 16KB / attn 16KB).
"""

import sys

for _p in ("/opt/trn_rl_repo", "/opt/trn_rl_repo/concourse"):
    if _p not in sys.path:
        sys.path.insert(0, _p)

import math

import ml_dtypes
import numpy as np

# ---------------------------------------------------------------- config
N_CORES = 8
NUM_HEADS = 16
ROPE_BASE = 10000.0
HD = 128  # head dim

MM_DT = "bfloat16"  # kept for test.py compat; kernel is bf16-only

_CACHE = {}

BF16 = ml_dtypes.bfloat16


def _full_cfg():
    return dict(B=2, S=2048, D=2048, NH=NUM_HEADS // N_CORES)


# ---------------------------------------------------------------- device program
def build_core_program(B, S, D, NH, mm_dt_name=None):
    """Build the single-core Bass program (identical on all 8 cores)."""
    import concourse.mybir as mybir
    from concourse import bacc
    from concourse.tile import TileContext

    f32 = mybir.dt.float32
    bf = mybir.dt.bfloat16

    hd = HD
    DQ = NH * hd           # per-core projection width (256)
    ET = D // 128          # contraction tiles over model dim
    SC = min(512, S)       # s-chunk width in projection phase (moving max)
    NSC = S // SC
    SBK = SC // 128        # s-blocks per chunk (for V)
    RU = min(512, S)       # RoPE unit width
    CPU = RU // SC         # chunks per rope unit
    SQT = min(512, S)      # attention sq tile width
    NSQ = S // SQT
    SKB = S // 128         # sk blocks
    NPAIR = SKB // 2
    SB = S // 128
    EOW = min(512, D)      # output-proj matmul width
    HW_ = D // 2           # output row half-width
    EOW2 = min(EOW, HW_)
    NEO2 = HW_ // EOW2
    inv_sqrt_hd = 1.0 / math.sqrt(hd)

    nc = bacc.Bacc(trn_type="TRN2", target_bir_lowering=False)

    xt = nc.dram_tensor("xt", [B, ET, 128, S], bf, kind="ExternalInput")
    wq = nc.dram_tensor("wq", [ET, 128, DQ], bf, kind="ExternalInput")
    wk = nc.dram_tensor("wk", [ET, 128, DQ], bf, kind="ExternalInput")
    wv = nc.dram_tensor("wv", [ET, 128, DQ], bf, kind="ExternalInput")
    wo = nc.dram_tensor("wo", [NH, 128, D], bf, kind="ExternalInput")
    cos = nc.dram_tensor("cos", [128, S], bf, kind="ExternalInput")
    sin = nc.dram_tensor("sin", [128, S], bf, kind="ExternalInput")
    rotm = nc.dram_tensor("rotm", [128, 128], bf, kind="ExternalInput")
    out = nc.dram_tensor("out", [B, SB, 128, D], bf, kind="ExternalOutput")

    Exp = mybir.ActivationFunctionType.Exp

    with TileContext(nc) as tc:
        with tc.tile_pool(name="sb", bufs=1) as sbp:
            # ---------- resident constants; weights/xt split into ETQ-row
            # groups (tile per group) to shorten first-matmul DMA waits
            ETQ = max(1, ET // 4)
            NG = ET // ETQ

            def wgroup_tiles(nm):
                return [
                    sbp.tile([128, ETQ, DQ], bf, name=f"{nm}_sb{g}")
                    for g in range(NG)
                ]

            wq_sb = wgroup_tiles("wq")
            wk_sb = wgroup_tiles("wk")
            wv_sb = wgroup_tiles("wv")
            wo_sb = sbp.tile([128, NH, D], bf, name="wo_sb")
            cos_sb = sbp.tile([128, S], bf, name="cos_sb")
            sin_sb = sbp.tile([128, S], bf, name="sin_sb")
            rot_sb = sbp.tile([128, 128], bf, name="rot_sb")
            ones_sb = sbp.tile([128, 128], bf, name="ones_sb")

            def load_xt_chunk(b, c):
                csl = slice(c * SC, (c + 1) * SC)
                tiles = []
                for g in range(NG):
                    t_sb = sbp.tile(
                        [128, ETQ, SC], bf, tag=f"xt{g}",
                        name=f"xt_{b}_{c}_{g}", bufs=2,
                    )
                    nc.sync.dma_start(
                        t_sb[:],
                        xt[b, g * ETQ : (g + 1) * ETQ, :, csl].rearrange(
                            "t p s -> p t s"
                        ),
                    )
                    tiles.append(t_sb)
                return tiles

            # All constant loads go on the sync queue: the Scalar queue must
            # stay free for the first projection copies (each DMA issue
            # costs ~0.6us of queue time)
            xt_next = load_xt_chunk(0, 0)
            # Q/K weights first (consumed from the first matmul); V weights
            # after (first needed at the end of chunk 0)
            for g in range(NG):
                for w_dram, w_tiles in ((wq, wq_sb), (wk, wk_sb)):
                    nc.scalar.dma_start(
                        w_tiles[g][:],
                        w_dram[g * ETQ : (g + 1) * ETQ].rearrange(
                            "t p d -> p t d"
                        ),
                    )
            for g in range(NG):
                nc.sync.dma_start(
                    wv_sb[g][:],
                    wv[g * ETQ : (g + 1) * ETQ].rearrange("t p d -> p t d"),
                )
            nc.sync.dma_start(cos_sb[:], cos[:])
            nc.sync.dma_start(sin_sb[:], sin[:])
            nc.sync.dma_start(rot_sb[:], rotm[:])
            ones_f32 = sbp.tile([128, 128], f32, name="ones_f32")
            nc.vector.memset(ones_f32[:], 1.0)
            nc.vector.tensor_copy(ones_sb[:], ones_f32[:])
            nc.sync.dma_start(wo_sb[:], wo[:].rearrange("h p e -> p h e"))

            def emit_wo(bb, at_list, pwp, pw_tag="pw", pw_bufs=2):
                # output projection for batch bb: half-D rows ([128, D/2]
                # PSUM accum -> ONE wide copy -> ONE DMA per half. Runs in
                # the attention scope, reusing the freed score-PSUM slots
                # (same 4KB slot size).
                for sb_i in range(SB):
                    for half in range(2):
                        pw = pwp.tile(
                            [128, HW_], f32, tag=pw_tag, bufs=pw_bufs
                        )
                        for eo in range(NEO2):
                            eosl = slice(
                                half * HW_ + eo * EOW2,
                                half * HW_ + (eo + 1) * EOW2,
                            )
                            qi_i = (sb_i * 128) // SQT
                            qoff = sb_i * 128 - qi_i * SQT
                            for a_t in range(NH):
                                nc.tensor.matmul(
                                    pw[:, eo * EOW2 : (eo + 1) * EOW2],
                                    at_list[a_t][qi_i][
                                        :, qoff : qoff + 128
                                    ],
                                    wo_sb[:, a_t, eosl],
                                    start=(a_t == 0),
                                    stop=(a_t == NH - 1),
                                )
                        osb = sbp.tile(
                            [128, HW_], bf, tag="osb", name="osb", bufs=6
                        )
                        # copies lean on Scalar (2 of 3): the DVE still
                        # drains the attention tail (recips/at-muls) when
                        # the wo phase starts
                        if half == 0 or sb_i < 5 or sb_i % 3 != 2:
                            nc.scalar.copy(osb[:], pw[:])
                        else:
                            nc.vector.tensor_copy(osb[:], pw[:])
                        nc.sync.dma_start(
                            out[bb, sb_i, :, half * HW_ : (half + 1) * HW_],
                            osb[:],
                        )

            at_prev = None
            for b in range(B):
                # ---------- projections + RoPE for batch b
                qt = [
                    sbp.tile([128, S], bf, tag=f"q{h}", name=f"qt{h}_{b}")
                    for h in range(NH)
                ]
                kt = [
                    sbp.tile([128, S], bf, tag=f"k{h}", name=f"kt{h}_{b}")
                    for h in range(NH)
                ]
                # V split into quarters to shorten tile-level dep chains
                SBV = max(1, SB // 4)
                v_sb = [
                    sbp.tile(
                        [128, SBV, DQ], bf, tag=f"v{g}", name=f"v{g}_{b}"
                    )
                    for g in range(SB // SBV)
                ]

                tasks = [(qi, h) for qi in range(NSQ) for h in range(NH)]
                allpairs = [
                    (ti, j)
                    for ti in range(len(tasks))
                    for j in range(NPAIR)
                ]
                DEPTH = 3

                with tc.tile_pool(name=f"pj{b}", bufs=1, space="PSUM") as pjp:
                    # previous batch's output projection shares this pool
                    # scope so no PSUM pool boundary stalls the PE
                    if at_prev is not None:
                        emit_wo(b - 1, at_prev, pjp)
                    raw = {}  # (h, 0=q/1=k) -> staging tile for current unit
                    for c in range(NSC):
                        csl_u = slice((c % CPU) * SC, (c % CPU + 1) * SC)
                        xt_sb = xt_next
                        nxt = (b, c + 1) if c + 1 < NSC else (b + 1, 0)
                        if nxt[0] < B:
                            xt_next = load_xt_chunk(*nxt)
                        if c % CPU == 0:
                            for h in range(NH):
                                raw[(h, 0)] = sbp.tile(
                                    [128, RU], bf, tag=f"qr{h}",
                                    name=f"qr{h}", bufs=2,
                                )
                                raw[(h, 1)] = sbp.tile(
                                    [128, RU], bf, tag=f"kr{h}",
                                    name=f"kr{h}", bufs=2,
                                )
                        for h in range(NH):
                            for i, w_sb in enumerate((wq_sb, wk_sb)):
                                ps = pjp.tile(
                                    [128, SC], f32, tag="pj", name="ps",
                                    bufs=2,
                                )
                                for t in range(ET):
                                    nc.tensor.matmul(
                                        ps[:],
                                        w_sb[t // ETQ][
                                            :, t % ETQ, h * hd : (h + 1) * hd
                                        ],
                                        xt_sb[t // ETQ][:, t % ETQ, :],
                                        start=(t == 0),
                                        stop=(t == ET - 1),
                                    )
                                nc.scalar.copy(raw[(h, i)][:, csl_u], ps[:])
                        for s2 in range(SBK):
                            psv = pjp.tile(
                                [128, DQ], f32, tag="pj", name="ps", bufs=2
                            )
                            for t in range(ET):
                                nc.tensor.matmul(
                                    psv[:],
                                    xt_sb[t // ETQ][
                                        :, t % ETQ, s2 * 128 : (s2 + 1) * 128
                                    ],
                                    wv_sb[t // ETQ][:, t % ETQ, :],
                                    start=(t == 0),
                                    stop=(t == ET - 1),
                                )
                            cb = c * SBK + s2
                            nc.scalar.copy(
                                v_sb[cb // SBV][:, cb % SBV, :], psv[:]
                            )
                        if (c + 1) % CPU == 0:
                            u = (c + 1) // CPU - 1
                            usl = slice(u * RU, (u + 1) * RU)
                            for h in range(NH):
                                for i, dst in ((0, qt[h]), (1, kt[h])):
                                    src = raw[(h, i)]
                                    prot = pjp.tile(
                                        [128, RU], f32, tag="rot",
                                        name="prot", bufs=2,
                                    )
                                    nc.tensor.matmul(
                                        prot[:], rot_sb[:], src[:],
                                        start=True, stop=True,
                                    )
                                    tsin = sbp.tile(
                                        [128, RU], bf, tag="tsin",
                                        name="tsin", bufs=2,
                                    )
                                    nc.vector.tensor_mul(
                                        tsin[:], prot[:], sin_sb[:, usl]
                                    )
                                    tcos = sbp.tile(
                                        [128, RU], bf, tag="tcos",
                                        name="tcos", bufs=2,
                                    )
                                    nc.vector.tensor_mul(
                                        tcos[:], src[:], cos_sb[:, usl]
                                    )
                                    nc.vector.tensor_add(
                                        dst[:, usl], tcos[:], tsin[:]
                                    )

                    # priming: first DEPTH pairs' scores+exp run here (their
                    # kt/qt units are long done), so the ACT pipeline is full
                    # when the attention scope opens
                    pend = []
                    for k in range(min(DEPTH, len(allpairs))):
                        tik, jk = allpairs[k]
                        qik, hk = tasks[tik]
                        sqk = slice(qik * SQT, (qik + 1) * SQT)
                        p_pr = sbp.tile(
                            [128, 2, SQT], bf, tag="p",
                            name=f"pp{tik}_{jk}", bufs=4,
                        )
                        for i in range(2):
                            ki = 2 * jk + i
                            scs = pjp.tile(
                                [128, SQT], f32, tag="pj", name="ps", bufs=2
                            )
                            nc.tensor.matmul(
                                scs[:],
                                kt[hk][:, ki * 128 : (ki + 1) * 128],
                                qt[hk][:, sqk],
                                start=True,
                                stop=True,
                            )
                            nc.scalar.activation(
                                p_pr[:, i, :], scs[:], Exp, scale=inv_sqrt_hd
                            )
                        pend.append(p_pr)

                # ---------- attention: flat pipeline over (qi, h, pair)
                # at split per (h, qi) so early wo row-blocks only wait
                # on their own qi's normalize, not the whole attention drain
                at = [
                    [
                        sbp.tile(
                            [128, SQT], bf, tag=f"a{h}_{qi}",
                            name=f"at{h}_{qi}_{b}",
                        )
                        for qi in range(NSQ)
                    ]
                    for h in range(NH)
                ]
                with tc.tile_pool(name=f"at{b}", bufs=1, space="PSUM") as app:
                    def score_pair(ti, j):
                        qi, h = tasks[ti]
                        sq = slice(qi * SQT, (qi + 1) * SQT)
                        sc_t = app.tile(
                            [128, 2, SQT], f32, tag="sc",
                            name=f"sc{ti}_{j}", bufs=2,
                        )
                        for i in range(2):
                            ki = 2 * j + i
                            nc.tensor.matmul(
                                sc_t[:, i, :],
                                kt[h][:, ki * 128 : (ki + 1) * 128],
                                qt[h][:, sq],
                                start=True,
                                stop=True,
                            )
                        p_sb = sbp.tile(
                            [128, 2, SQT], bf, tag="p",
                            name=f"p{ti}_{j}", bufs=4,
                        )
                        nc.scalar.activation(
                            p_sb[:], sc_t[:], Exp, scale=inv_sqrt_hd
                        )
                        return p_sb

                    po = pr = None
                    for idx, (ti, j) in enumerate(allpairs):
                        qi, h = tasks[ti]
                        sq = slice(qi * SQT, (qi + 1) * SQT)
                        p_sb = pend.pop(0)
                        if idx + DEPTH < len(allpairs):
                            pend.append(score_pair(*allpairs[idx + DEPTH]))
                        if j == 0:
                            po = app.tile(
                                [128, SQT], f32, tag="oc", name="po", bufs=2
                            )
                            pr = app.tile(
                                [128, SQT], f32, tag="rc", name="pr", bufs=2
                            )
                        for i in range(2):
                            ki = 2 * j + i
                            nc.tensor.matmul(
                                po[:],
                                v_sb[ki // SBV][
                                    :, ki % SBV, h * hd : (h + 1) * hd
                                ],
                                p_sb[:, i, :],
                                start=(ki == 0),
                                stop=(ki == SKB - 1),
                            )
                            nc.tensor.matmul(
                                pr[:],
                                ones_sb[:],
                                p_sb[:, i, :],
                                start=(ki == 0),
                                stop=(ki == SKB - 1),
                            )
                        if j == NPAIR - 1:
                            r_sb = sbp.tile(
                                [128, SQT], f32, tag="r", name="r_sb", bufs=2
                            )
                            nc.vector.reciprocal_approx_fast(
                                out=r_sb[:], in_=pr[:]
                            )
                            nc.vector.tensor_mul(
                                at[h][qi][:], po[:], r_sb[:]
                            )

                at_prev = at

            # final batch's output projection (whole PSUM is free here)
            with tc.tile_pool(name="wolast", bufs=1, space="PSUM") as pwp:
                emit_wo(B - 1, at_prev, pwp, pw_bufs=4)

    nc.compile()
    return nc


# ---------------------------------------------------------------- host helpers
def _rope_tables(S, dtype=BF16):
    """cos/sin tables [128, S] in [d, s] layout (plain sin; sign lives in
    the rotation matrix)."""
    inv_freq = 1.0 / (ROPE_BASE ** (np.arange(0, HD, 2, dtype=np.float32) / HD))
    t = np.arange(S, dtype=np.float32)
    freqs = np.outer(t, inv_freq)  # [S, half]
    cos = np.cos(freqs).T  # [half, S]
    sin = np.sin(freqs).T
    cosT = np.concatenate([cos, cos], axis=0).astype(dtype)  # [128, S]
    sinT = np.concatenate([sin, sin], axis=0).astype(dtype)
    return np.ascontiguousarray(cosT), np.ascontiguousarray(sinT)


def _rot_matrix(dtype=BF16):
    """Signed permutation R [128,128] (stationary layout) s.t.
    (R^T q)[i] = rotate_half(q)[i] for q in [d, s] layout."""
    half = HD // 2
    m = np.zeros((HD, HD), dtype=np.float32)
    for i in range(HD):
        m[(i + half) % HD, i] = -1.0 if i < half else 1.0
    return np.ascontiguousarray(m.astype(dtype))


def _prep_inputs(hidden_states, Wq, Wk, Wv, Wo, cfg, n_cores=N_CORES):
    """Build the per-core input dicts (all bf16)."""
    B, S, D, NH = cfg["B"], cfg["S"], cfg["D"], cfg["NH"]
    ET = D // 128
    DQ = NH * HD

    x = np.asarray(hidden_states, dtype=np.float32)
    xt = (
        np.ascontiguousarray(x.transpose(0, 2, 1))
        .astype(BF16)
        .reshape(B, ET, 128, S)
    )
    cosT, sinT = _rope_tables(S)
    rotmat = _rot_matrix()

    in_maps = []
    for c in range(n_cores):
        lo, hi = c * DQ, (c + 1) * DQ
        wq_c = np.ascontiguousarray(np.asarray(Wq)[lo:hi, :].T).astype(BF16)
        wk_c = np.ascontiguousarray(np.asarray(Wk)[lo:hi, :].T).astype(BF16)
        wv_c = np.ascontiguousarray(np.asarray(Wv)[lo:hi, :].T).astype(BF16)
        wo_c = np.ascontiguousarray(np.asarray(Wo)[:, lo:hi].T).astype(BF16)
        in_maps.append(
            {
                "xt": xt,
                "wq": wq_c.reshape(ET, 128, DQ),
                "wk": wk_c.reshape(ET, 128, DQ),
                "wv": wv_c.reshape(ET, 128, DQ),
                "wo": wo_c.reshape(NH, 128, D),
                "cos": cosT,
                "sin": sinT,
                "rotm": rotmat,
            }
        )
    return in_maps


def _gather(results, cfg):
    B, S, D = cfg["B"], cfg["S"], cfg["D"]
    acc = np.zeros((B, S, D), dtype=np.float64)
    for r in results:
        acc += np.asarray(r["out"]).astype(np.float64).reshape(B, S, D)
    return acc.astype(np.float32)


# ---------------------------------------------------------------- entry point
def kernel(hidden_states, Wq, Wk, Wv, Wo):
    from concourse.bass_utils import run_bass_kernel_spmd

    cfg = _full_cfg()
    key = ("nc", cfg["B"], cfg["S"], cfg["D"], cfg["NH"])
    if key not in _CACHE:
        _CACHE[key] = build_core_program(cfg["B"], cfg["S"], cfg["D"], cfg["NH"])
    nc = _CACHE[key]

    in_maps = _prep_inputs(hidden_states, Wq, Wk, Wv, Wo, cfg)
    res = run_bass_kernel_spmd(nc, in_maps, core_ids=list(range(N_CORES)))
    return _gather(res.results, cfg)
